# revision 1
# baseline (speedup 1.0000x reference)
"""Grouped SwiGLU experts (MoE post-dispatch compute) on 8 Trainium2 cores.

Expert-parallel: host gathers tokens per expert (the "all-to-all dispatch"),
packs 128-token blocks into a uniform per-core segment schedule (specialized
to the actual counts at compile time), each segment runs one expert's SwiGLU
  hT = silu(w1.T x.T) * (w3.T x.T);  out = (hT.T) @ w2
on one NeuronCore in bf16 with fp32 PSUM accumulation, and the host
scatters rows back to their original token positions.
"""

import numpy as np
import ml_dtypes

# ---- problem constants (from the reference module) ----
T, D, H, E, R, ALIGN = 8192, 4096, 1024, 8, 2, 16
P = 128          # partition width
DT = D // P      # 32 d-tiles
HT = H // P      # 8 h-chunks
NCORES = 8
UNIT = 16        # scheduling granularity in rows (reference ALIGN)
MAX_UNITS = 32   # max units per segment = 512 rows (psum bank / matmul N cap)

BF16 = ml_dtypes.bfloat16


def _permute_indices(counts):
    """numpy port of reference._permute_indices."""
    counts = counts.astype(np.int64)
    max_len = T + E * ALIGN
    start_index = np.cumsum(counts) - counts
    total = counts.reshape(R, E).sum(0)
    m_sizes = ((np.maximum(total, ALIGN) + ALIGN - 1) // ALIGN * ALIGN).astype(np.int64)
    m_offsets = np.cumsum(m_sizes)
    write_offsets = m_offsets - m_sizes
    c_er = counts.reshape(R, E).T
    seg_ws = (write_offsets[:, None] + np.cumsum(c_er, 1) - c_er).reshape(-1)
    seg_len = c_er.reshape(-1)
    seg_src = start_index.reshape(R, E).T.reshape(-1)
    pos = np.arange(max_len, dtype=np.int64)
    idx = np.clip(np.searchsorted(seg_ws, pos, side="right") - 1, 0, E * R - 1)
    within = pos - seg_ws[idx]
    valid = (within >= 0) & (within < seg_len[idx])
    perm = np.where(valid, seg_src[idx] + within, T)
    return perm.astype(np.int64), m_sizes, (m_offsets - m_sizes)


def _partitions(c, max_part, max_len):
    """Partitions of c into <= max_len parts each <= max_part, desc order."""
    out = []

    def rec(rem, mx, cur):
        if rem == 0:
            out.append(tuple(cur))
            return
        if len(cur) == max_len:
            return
        for p in range(min(rem, mx), 0, -1):
            cur.append(p)
            rec(rem - p, p, cur)
            cur.pop()

    rec(c, max_part, [])
    return out


def _ffd(nblk, parts):
    """Pack per-expert block counts into 8 cores x len(parts) bins
    (bin (c,j) capacity parts[j], single expert per bin). Returns
    assignment dict (core, j) -> (expert, block_start, nb) or None."""
    bins = []  # (cap, core, j)
    for c in range(NCORES):
        for j, cap in enumerate(parts):
            bins.append([cap, c, j])
    bins.sort(key=lambda b: -b[0])
    used = [False] * len(bins)
    asg = {}
    order = sorted(range(E), key=lambda e: -nblk[e])
    for e in order:
        rem = int(nblk[e])
        b0 = 0
        while rem > 0:
            # largest unused bin with cap <= rem (fill fully); else the
            # smallest unused bin (minimize slack)
            best_le, best_gt = None, None
            for i, (cap, c, j) in enumerate(bins):
                if used[i]:
                    continue
                if cap <= rem:
                    best_le = i  # bins desc: first such is largest
                    break
                best_gt = i  # keeps updating: last seen = smallest so far
            i = best_le if best_le is not None else best_gt
            if i is None:
                return None
            cap, c, j = bins[i]
            used[i] = True
            nb = min(cap, rem)
            asg[(c, j)] = (e, b0, nb)
            b0 += nb
            rem -= nb
    return asg


def _plan(nblk):
    """nblk: per-expert sizes in UNIT-row units. Returns (parts, asg) with
    parts = per-segment capacities in units (same on all cores)."""
    n = int(sum(nblk))
    c0 = max(1, -(-n // NCORES))
    # a segment costs a full weight stream (~25MB) regardless of size, so
    # prefer <=2 segments even at a few extra rows before allowing 3
    for max_len, c_hi in ((2, c0 + 8), (3, c0 + 8), (3, 600)):
        for c in range(c0, c_hi):
            opts = _partitions(c, MAX_UNITS, max_len)
            # most balanced first (largest min part)
            for parts in sorted(opts, key=lambda p: (len(p), -min(p))):
                asg = _ffd(nblk, parts)
                if asg is not None:
                    return list(parts), asg
    raise RuntimeError("packing failed")


def _build_program(parts):
    import concourse.mybir as mybir
    import concourse.tile as tile
    from concourse import bacc

    bf = mybir.dt.bfloat16
    f32 = mybir.dt.float32
    SILU = mybir.ActivationFunctionType.Silu

    nc = bacc.Bacc("TRN2", target_bir_lowering=False, debug=False,
                   num_devices=NCORES)

    xt_d, w1_d, w3_d, w2_d, out_d = [], [], [], [], []
    for j, U in enumerate(parts):
        M = U * UNIT
        xt_d.append(nc.dram_tensor(f"xt{j}", [DT, P, M], bf, kind="ExternalInput"))
        w1_d.append(nc.dram_tensor(f"w1p{j}", [HT, P, D], bf, kind="ExternalInput"))
        w3_d.append(nc.dram_tensor(f"w3p{j}", [HT, P, D], bf, kind="ExternalInput"))
        w2_d.append(nc.dram_tensor(f"w2p{j}", [H, D], bf, kind="ExternalInput"))
        out_d.append(nc.dram_tensor(f"out{j}", [M, D], bf, kind="ExternalOutput"))

    with tile.TileContext(nc) as tc:
        with (
            tc.tile_pool(name="xt", bufs=2 * DT) as xt_pool,
            tc.tile_pool(name="wp", bufs=10) as wp_pool,
            tc.tile_pool(name="w2", bufs=10) as w2_pool,
            tc.tile_pool(name="ht", bufs=2 * HT) as ht_pool,
            tc.tile_pool(name="stmp", bufs=2) as stmp_pool,
            tc.tile_pool(name="ost", bufs=6) as ost_pool,
            tc.tile_pool(name="ps1", bufs=2, space="PSUM") as ps1_pool,
            tc.tile_pool(name="ps3", bufs=2, space="PSUM") as ps3_pool,
            tc.tile_pool(name="pso", bufs=3, space="PSUM") as pso_pool,
            tc.tile_pool(name="warm", bufs=1) as warm_pool,
            tc.tile_pool(name="pwarm", bufs=1, space="PSUM") as pwarm_pool,
        ):
            # keep the PE busy (HAM at K=8/8) while the first real DMAs land
            wz = warm_pool.tile([P, P], bf, tag="warm", name="warmz")
            nc.gpsimd.memset(wz[:], 0.0)
            pw = pwarm_pool.tile([P, P], f32, tag="pwarm", name="warmp")
            for _ in range(140):
                nc.tensor.matmul(pw[:], wz[:], wz[:], start=True, stop=True)
            for j, U in enumerate(parts):
                M = U * UNIT

                def load_wp(h, j=j):
                    # half-panels: first matmuls only wait on 512KB, and
                    # panel loads pipeline at finer grain
                    DH = D // 2
                    tiles = []
                    for half in range(2):
                        t1 = wp_pool.tile([P, DH], bf, tag="wp",
                                          name=f"w1p{j}_{h}_{half}")
                        nc.sync.dma_start(
                            out=t1[:], in_=w1_d[j][h, :, half * DH:(half + 1) * DH])
                        t3 = wp_pool.tile([P, DH], bf, tag="wp",
                                          name=f"w3p{j}_{h}_{half}")
                        nc.sync.dma_start(
                            out=t3[:], in_=w3_d[j][h, :, half * DH:(half + 1) * DH])
                        tiles.append((t1, t3))
                    return tiles

                # critical path first: h=0 weight panels, then token tiles;
                # w2 (phase 2 only) is deferred until after phase 1 emission
                wp_cur = load_wp(0)
                xts = []
                for d in range(DT):
                    t = xt_pool.tile([P, M], bf, tag="xt", name=f"xt{j}_{d}")
                    nc.sync.dma_start(out=t[:], in_=xt_d[j][d])
                    xts.append(t)
                hts = [ht_pool.tile([P, M], bf, tag="ht", name=f"ht{j}_{h}")
                       for h in range(HT)]
                for h in range(HT):
                    wp_halves = wp_cur
                    if h + 1 < HT:
                        wp_cur = load_wp(h + 1)
                    ps1 = ps1_pool.tile([P, M], f32, tag="ps1")
                    ps3 = ps3_pool.tile([P, M], f32, tag="ps3")
                    DH = DT // 2
                    for d in range(DT):
                        w1p, w3p = wp_halves[d // DH]
                        dd = d % DH
                        nc.tensor.matmul(ps1[:], w1p[:, dd * P:(dd + 1) * P],
                                         xts[d][:], start=(d == 0),
                                         stop=(d == DT - 1))
                        nc.tensor.matmul(ps3[:], w3p[:, dd * P:(dd + 1) * P],
                                         xts[d][:], start=(d == 0),
                                         stop=(d == DT - 1))
                    tmp = stmp_pool.tile([P, M], f32, tag="stmp")
                    nc.scalar.activation(tmp[:], ps1[:], SILU)
                    nc.vector.tensor_mul(hts[h][:], tmp[:], ps3[:])
                w2s = []
                for h in range(HT):
                    t = w2_pool.tile([P, D], bf, tag="w2", name=f"w2{j}_{h}")
                    nc.sync.dma_start(out=t[:], in_=w2_d[j][h * P:(h + 1) * P, :])
                    w2s.append(t)
                NB = (M + P - 1) // P
                for b in range(NB):
                    pb = min(P, M - b * P)  # last block may be partial
                    for dc in range(D // 512):
                        po = pso_pool.tile([P, 512], f32, tag="pso")
                        for h in range(HT):
                            nc.tensor.matmul(
                                po[:pb, :], hts[h][:, b * P:b * P + pb],
                                w2s[h][:, dc * 512:(dc + 1) * 512],
                                start=(h == 0), stop=(h == HT - 1))
                        ob = ost_pool.tile([P, 512], bf, tag="ost")
                        nc.vector.tensor_copy(ob[:pb, :], po[:pb, :])
                        nc.sync.dma_start(
                            out=out_d[j][b * P:b * P + pb, dc * 512:(dc + 1) * 512],
                            in_=ob[:pb, :])

    nc.compile()
    return nc


_CACHE = {}


def _get_program(parts):
    key = tuple(parts)
    if key not in _CACHE:
        _CACHE[key] = _build_program(parts)
    return _CACHE[key]


_LAST_RESULT = None


def kernel(x, w1, w2, w3, num_tokens_per_expert):
    import os
    from concourse.bass_utils import run_bass_kernel_spmd

    x = np.asarray(x, dtype=np.float32)
    counts = np.asarray(num_tokens_per_expert).astype(np.int64)
    perm, m_sizes, m_off = _permute_indices(counts)
    nunits = m_sizes // UNIT  # m_sizes are UNIT-aligned

    parts, asg = _plan(nunits)
    nc = _get_program(parts)

    # expert-grouped token stream (the dispatch): rows of x per expert
    x_pad = np.concatenate([x, np.zeros((1, D), np.float32)], axis=0)
    ltot = int(m_sizes.sum())
    xp = x_pad[perm[:ltot]]  # [ltot, D] expert-grouped, 16-aligned per expert
    xe = [xp[m_off[e]:m_off[e] + m_sizes[e]] for e in range(E)]

    w1b = [np.ascontiguousarray(
        np.asarray(w1[e], np.float32).reshape(DT, P, HT, P)
        .transpose(2, 1, 0, 3).reshape(HT, P, D)).astype(BF16) for e in range(E)]
    w3b = [np.ascontiguousarray(
        np.asarray(w3[e], np.float32).reshape(DT, P, HT, P)
        .transpose(2, 1, 0, 3).reshape(HT, P, D)).astype(BF16) for e in range(E)]
    w2b = [np.asarray(w2[e], np.float32).astype(BF16) for e in range(E)]

    in_maps = []
    for c in range(NCORES):
        m = {}
        for j, U in enumerate(parts):
            M = U * UNIT
            ent = asg.get((c, j))
            e = ent[0] if ent is not None else 0
            blk = np.zeros((M, D), np.float32)
            if ent is not None:
                _, u0, nu = ent
                blk[:nu * UNIT] = xe[e][u0 * UNIT:(u0 + nu) * UNIT]
            # [DT, P, M]: xt[d, p, m] = blk[m, d*P+p]
            m[f"xt{j}"] = np.ascontiguousarray(
                blk.reshape(M, DT, P).transpose(1, 2, 0)).astype(BF16)
            m[f"w1p{j}"] = w1b[e]
            m[f"w3p{j}"] = w3b[e]
            m[f"w2p{j}"] = w2b[e]
        in_maps.append(m)

    kw = {}
    if os.environ.get("KERNEL_TRACE"):
        kw = dict(trace=True, tmpdir=os.environ.get("KERNEL_TRACE_DIR") or None)
    res = run_bass_kernel_spmd(nc, in_maps, core_ids=list(range(NCORES)), **kw)
    global _LAST_RESULT
    _LAST_RESULT = res

    # reassemble expert-grouped output stream, then scatter to token order
    outp = np.zeros((ltot, D), np.float32)
    for (c, j), (e, u0, nu) in asg.items():
        nr = nu * UNIT
        outp[m_off[e] + u0 * UNIT:m_off[e] + u0 * UNIT + nr] = \
            res.results[c][f"out{j}"][:nr]

    out = np.zeros((T + 1, D), np.float32)
    out[perm[:ltot]] = outp
    return out[:T]



# revision 2
# speedup vs baseline: 1.1026x; 1.1026x over previous
"""Grouped SwiGLU experts (MoE post-dispatch compute) on 8 Trainium2 cores.

Expert-parallel with optional hidden-dim tensor parallelism: host gathers
tokens per expert (the "all-to-all dispatch") and packs them into a uniform
per-core slot schedule (specialized to the actual counts at compile time).
A slot is a weight stream for one expert covering either the full hidden dim
(8 h-chunks) or half of it (4 h-chunks); half-width slots halve the weight
traffic per core and their partial outputs are summed on the host. Each slot
runs  hT = silu(w1.T x.T) * (w3.T x.T);  out = (hT.T) @ w2  in bf16 with
fp32 PSUM accumulation, and the host scatters rows back to token positions.
"""

import itertools
import numpy as np
import ml_dtypes

# ---- problem constants (from the reference module) ----
T, D, H, E, R, ALIGN = 8192, 4096, 1024, 8, 2, 16
P = 128          # partition width
DT = D // P      # 32 d-tiles
HT = H // P      # 8 h-chunks (full width)
NCORES = 8
UNIT = 16        # scheduling granularity in rows (reference ALIGN)
MAX_UNITS = 32   # max rows per slot = 512 (psum bank / f32 free-dim cap)

BF16 = ml_dtypes.bfloat16

# cost model for the planner
HU_NS = 2560          # compute ns per half-unit (16 rows x 384 cyc @2.4GHz)
PLAN_BW = 310.0       # planning DMA bandwidth, bytes/ns
W_FULL = 3 * D * H * 2            # full weight stream bytes (25.2MB)
W_HALF = W_FULL // 2
ROW_IO = 2 * D * 2                # x + out bytes per scheduled row (bf16)


def _permute_indices(counts):
    """numpy port of reference._permute_indices."""
    counts = counts.astype(np.int64)
    max_len = T + E * ALIGN
    start_index = np.cumsum(counts) - counts
    total = counts.reshape(R, E).sum(0)
    m_sizes = ((np.maximum(total, ALIGN) + ALIGN - 1) // ALIGN * ALIGN).astype(np.int64)
    m_offsets = np.cumsum(m_sizes)
    write_offsets = m_offsets - m_sizes
    c_er = counts.reshape(R, E).T
    seg_ws = (write_offsets[:, None] + np.cumsum(c_er, 1) - c_er).reshape(-1)
    seg_len = c_er.reshape(-1)
    seg_src = start_index.reshape(R, E).T.reshape(-1)
    pos = np.arange(max_len, dtype=np.int64)
    idx = np.clip(np.searchsorted(seg_ws, pos, side="right") - 1, 0, E * R - 1)
    within = pos - seg_ws[idx]
    valid = (within >= 0) & (within < seg_len[idx])
    perm = np.where(valid, seg_src[idx] + within, T)
    return perm.astype(np.int64), m_sizes, (m_offsets - m_sizes)


def _ffd(tasks, bins, strict):
    """Pack tasks (key, size) into bins [cap, core, j], splitting freely.
    Single task piece per bin. Returns (asg {(core,j): (key, u0, nu)},
    leftovers {key: units}); if strict, returns None on leftover."""
    bins = sorted(bins, key=lambda b: -b[0])
    used = [False] * len(bins)
    asg = {}
    left = {}
    for key, size in sorted(tasks, key=lambda t: -t[1]):
        rem = int(size)
        u0 = 0
        while rem > 0:
            best_le, best_gt = None, None
            for i, (cap, c, j) in enumerate(bins):
                if used[i]:
                    continue
                if cap <= rem:
                    best_le = i  # bins desc: first such is largest
                    break
                best_gt = i  # keeps updating: last seen = smallest so far
            i = best_le if best_le is not None else best_gt
            if i is None:
                if strict:
                    return None
                left[key] = rem
                break
            cap, c, j = bins[i]
            used[i] = True
            nb = min(cap, rem)
            asg[(c, j)] = (key, u0, nb)
            u0 += nb
            rem -= nb
    return asg, left


def _try_config(slots, nunits):
    """slots: [(units, nhc)]. Returns asg {(c,j): (e, half, u0, nu)} with
    half=None for full-width slots, or None if infeasible."""
    full_bins = [[u, c, j] for c in range(NCORES)
                 for j, (u, nhc) in enumerate(slots) if nhc == HT]
    half_bins = [[u, c, j] for c in range(NCORES)
                 for j, (u, nhc) in enumerate(slots) if nhc == HT // 2]
    full_tasks = [(e, int(nunits[e])) for e in range(E)]
    if full_bins:
        r = _ffd(full_tasks, full_bins, strict=False)
        asg_f, left = r
    else:
        asg_f, left = {}, {e: int(nunits[e]) for e in range(E)}
    if left and not half_bins:
        return None
    # leftover of expert e needs BOTH halves scheduled (independently split)
    half_tasks = [((e, h), r) for e, r in left.items() for h in (0, 1)]
    r = _ffd(half_tasks, half_bins, strict=True) if half_tasks else ({}, {})
    if r is None:
        return None
    asg_h, _ = r
    # full-stage pieces start after... full and half pieces of expert e cover
    # disjoint unit ranges: full pieces cover [0, nf_e), halves [nf_e, n_e)
    nf = {e: 0 for e in range(E)}
    for (c, j), (e, u0, nu) in asg_f.items():
        nf[e] = max(nf[e], u0 + nu)
    asg = {}
    for (c, j), (e, u0, nu) in asg_f.items():
        asg[(c, j)] = (e, None, u0, nu)
    for (c, j), ((e, h), u0, nu) in asg_h.items():
        asg[(c, j)] = (e, h, nf[e] + u0, nu)
    return asg


def _cost(slots):
    chu = sum(u * (2 if nhc == HT else 1) for u, nhc in slots)
    rows = sum(u for u, _ in slots) * UNIT
    wbytes = sum(W_FULL if nhc == HT else W_HALF for _, nhc in slots)
    dma_ns = (wbytes + rows * ROW_IO) / PLAN_BW + 8000
    comp_ns = chu * HU_NS + 12000
    return max(comp_ns, dma_ns), dma_ns, len(slots)


def _plan(nunits):
    """Returns (slots [(units, nhc)], asg {(c,j): (e, half, u0, nu)})."""
    total_hu = 2 * int(sum(nunits))
    need = -(-total_hu // NCORES)
    best = None
    for extra_cap in (10, 40, 2 * need):
        for ns in (1, 2, 3):
            for widths in itertools.product((HT, HT // 2), repeat=ns):
                for us in itertools.product(range(1, MAX_UNITS + 1), repeat=ns):
                    cfg = tuple(sorted(zip(us, widths), key=lambda s: (-s[1], -s[0])))
                    if cfg != tuple(zip(us, widths)):
                        continue  # canonical order only
                    chu = sum(u * (2 if w == HT else 1) for u, w in cfg)
                    if not (need <= chu <= need + extra_cap):
                        continue
                    c = _cost(cfg)
                    if best is not None and c >= best[0]:
                        continue
                    asg = _try_config(cfg, nunits)
                    if asg is not None:
                        best = (c, list(cfg), asg)
        if best is not None:
            break
    if best is None:
        raise RuntimeError("packing failed")
    return best[1], best[2]


def _build_program(slots):
    import concourse.mybir as mybir
    import concourse.tile as tile
    from concourse import bacc

    bf = mybir.dt.bfloat16
    f32 = mybir.dt.float32
    SILU = mybir.ActivationFunctionType.Silu
    COPY = mybir.ActivationFunctionType.Copy

    nc = bacc.Bacc("TRN2", target_bir_lowering=False, debug=False,
                   num_devices=NCORES)

    XC = DT // 4  # d-tiles per xt chunk

    xt_d, w13_d, w2_d, out_d = [], [], [], []
    for j, (U, NHC) in enumerate(slots):
        M = U * UNIT
        NB = (M + P - 1) // P
        xt_d.append(nc.dram_tensor(f"xt{j}", [4, P, XC * M], bf, kind="ExternalInput"))
        w13_d.append(nc.dram_tensor(f"w13p{j}", [2, NHC, P, D], bf, kind="ExternalInput"))
        w2_d.append(nc.dram_tensor(f"w2p{j}", [NHC, P, D], bf, kind="ExternalInput"))
        out_d.append(nc.dram_tensor(f"out{j}", [NB, P, D], bf, kind="ExternalOutput"))

    with tile.TileContext(nc) as tc:
        with (
            tc.tile_pool(name="xt", bufs=8) as xt_pool,
            tc.tile_pool(name="wp", bufs=4) as wp_pool,
            tc.tile_pool(name="w2", bufs=10) as w2_pool,
            tc.tile_pool(name="ht", bufs=12) as ht_pool,
            tc.tile_pool(name="stmp", bufs=2) as stmp_pool,
            tc.tile_pool(name="ost", bufs=2) as ost_pool,
            tc.tile_pool(name="ps1", bufs=2, space="PSUM") as ps1_pool,
            tc.tile_pool(name="ps3", bufs=2, space="PSUM") as ps3_pool,
            tc.tile_pool(name="pso", bufs=2, space="PSUM") as pso_pool,
            tc.tile_pool(name="warm", bufs=1) as warm_pool,
            tc.tile_pool(name="pwarm", bufs=1, space="PSUM") as pwarm_pool,
        ):
            # keep the PE busy (HAM at K=8/8) while the first real DMAs land
            wz = warm_pool.tile([P, P], bf, tag="warm", name="warmz")
            nc.gpsimd.memset(wz[:], 0.0)
            pw = pwarm_pool.tile([P, P], f32, tag="pwarm", name="warmp")
            for _ in range(140):
                nc.tensor.matmul(pw[:], wz[:], wz[:], start=True, stop=True)

            for j, (U, NHC) in enumerate(slots):
                M = U * UNIT
                NB = (M + P - 1) // P

                def load_wpair(hc, j=j):
                    t1 = wp_pool.tile([P, D], bf, tag="wp", name=f"w1_{j}_{hc}")
                    nc.sync.dma_start(out=t1[:], in_=w13_d[j][0, hc])
                    t3 = wp_pool.tile([P, D], bf, tag="wp", name=f"w3_{j}_{hc}")
                    nc.sync.dma_start(out=t3[:], in_=w13_d[j][1, hc])
                    return (t1, t3)

                # critical path first: hc=0 weight panels, then token chunks;
                # w2 (phase 2 only) is deferred until after hc=0 emission
                wpair = load_wpair(0)
                xts = []
                for k in range(4):
                    t = xt_pool.tile([P, XC * M], bf, tag="xt", name=f"xt{j}_{k}")
                    nc.sync.dma_start(out=t[:], in_=xt_d[j][k])
                    xts.append(t)
                hts = [ht_pool.tile([P, M], bf, tag="ht", name=f"ht{j}_{h}")
                       for h in range(NHC)]
                w2s = None
                for hc in range(NHC):
                    wcur = wpair
                    if hc + 1 < NHC:
                        wpair = load_wpair(hc + 1)
                    ps1 = ps1_pool.tile([P, M], f32, tag="ps1")
                    ps3 = ps3_pool.tile([P, M], f32, tag="ps3")
                    for d in range(DT):
                        xa = xts[d // XC][:, (d % XC) * M:(d % XC + 1) * M]
                        nc.tensor.matmul(ps1[:], wcur[0][:, d * P:(d + 1) * P],
                                         xa, start=(d == 0), stop=(d == DT - 1))
                        nc.tensor.matmul(ps3[:], wcur[1][:, d * P:(d + 1) * P],
                                         xa, start=(d == 0), stop=(d == DT - 1))
                    tmp = stmp_pool.tile([P, M], f32, tag="stmp")
                    nc.scalar.activation(tmp[:], ps1[:], SILU)
                    nc.vector.tensor_mul(hts[hc][:], tmp[:], ps3[:])
                    if hc == 0:
                        w2s = []
                        for h in range(NHC):
                            t = w2_pool.tile([P, D], bf, tag="w2",
                                             name=f"w2_{j}_{h}")
                            nc.sync.dma_start(out=t[:], in_=w2_d[j][h])
                            w2s.append(t)
                for b in range(NB):
                    pb = min(P, M - b * P)  # last block may be partial
                    ob = ost_pool.tile([P, D], bf, tag="ost")
                    for dc in range(D // 512):
                        po = pso_pool.tile([P, 512], f32, tag="pso")
                        for h in range(NHC):
                            nc.tensor.matmul(
                                po[:pb, :], hts[h][:, b * P:b * P + pb],
                                w2s[h][:, dc * 512:(dc + 1) * 512],
                                start=(h == 0), stop=(h == NHC - 1))
                        # split psum->sbuf casts across scalar and vector
                        if dc % 2 == 0:
                            nc.scalar.activation(
                                ob[:pb, dc * 512:(dc + 1) * 512], po[:pb, :], COPY)
                        else:
                            nc.vector.tensor_copy(
                                ob[:pb, dc * 512:(dc + 1) * 512], po[:pb, :])
                    nc.sync.dma_start(out=out_d[j][b, :pb, :], in_=ob[:pb, :])

    nc.compile()
    return nc


_CACHE = {}


def _get_program(slots):
    key = tuple(slots)
    if key not in _CACHE:
        _CACHE[key] = _build_program(slots)
    return _CACHE[key]


_LAST_RESULT = None


def kernel(x, w1, w2, w3, num_tokens_per_expert):
    import os
    from concourse.bass_utils import run_bass_kernel_spmd

    x = np.asarray(x, dtype=np.float32)
    counts = np.asarray(num_tokens_per_expert).astype(np.int64)
    perm, m_sizes, m_off = _permute_indices(counts)
    nunits = m_sizes // UNIT  # m_sizes are UNIT-aligned

    slots, asg = _plan(nunits)
    nc = _get_program(slots)

    # expert-grouped token stream (the dispatch): rows of x per expert
    x_pad = np.concatenate([x, np.zeros((1, D), np.float32)], axis=0)
    ltot = int(m_sizes.sum())
    xp = x_pad[perm[:ltot]].astype(BF16)  # [ltot, D] expert-grouped
    xe = [xp[m_off[e]:m_off[e] + m_sizes[e]] for e in range(E)]

    w1b = [np.ascontiguousarray(
        np.asarray(w1[e], np.float32).reshape(DT, P, HT, P)
        .transpose(2, 1, 0, 3).reshape(HT, P, D)).astype(BF16) for e in range(E)]
    w3b = [np.ascontiguousarray(
        np.asarray(w3[e], np.float32).reshape(DT, P, HT, P)
        .transpose(2, 1, 0, 3).reshape(HT, P, D)).astype(BF16) for e in range(E)]
    w2b = [np.asarray(w2[e], np.float32).astype(BF16).reshape(HT, P, D)
           for e in range(E)]

    XC = DT // 4
    w13_cache = {}

    def w13_for(e, half, nhc):
        key = (e, half)
        if key not in w13_cache:
            off = 0 if half is None else half * nhc
            w13_cache[key] = np.stack([w1b[e][off:off + nhc],
                                       w3b[e][off:off + nhc]])
        return w13_cache[key]

    in_maps = []
    for c in range(NCORES):
        mm = {}
        for j, (U, NHC) in enumerate(slots):
            M = U * UNIT
            ent = asg.get((c, j))
            e, half = (ent[0], ent[1]) if ent is not None else (0, None if NHC == HT else 0)
            blk = np.zeros((M, D), BF16)
            if ent is not None:
                _, _, u0, nu = ent
                blk[:nu * UNIT] = xe[e][u0 * UNIT:(u0 + nu) * UNIT]
            # xt[k][p, t*M+m] = blk[m, (k*XC+t)*128+p]
            mm[f"xt{j}"] = np.ascontiguousarray(
                blk.reshape(M, 4, XC, P).transpose(1, 3, 2, 0).reshape(4, P, XC * M))
            mm[f"w13p{j}"] = w13_for(e, half, NHC)
            off = 0 if half is None else half * NHC
            mm[f"w2p{j}"] = w2b[e][off:off + NHC]
        in_maps.append(mm)

    kw = {}
    if os.environ.get("KERNEL_TRACE"):
        kw = dict(trace=True, tmpdir=os.environ.get("KERNEL_TRACE_DIR") or None)
    res = run_bass_kernel_spmd(nc, in_maps, core_ids=list(range(NCORES)), **kw)
    global _LAST_RESULT
    _LAST_RESULT = res

    # reassemble expert-grouped output stream (summing half partials),
    # then scatter to token order
    outp = np.zeros((ltot, D), np.float32)
    for (c, j), (e, half, u0, nu) in asg.items():
        nr = nu * UNIT
        M = slots[j][0] * UNIT
        NB = (M + P - 1) // P
        seg = np.asarray(res.results[c][f"out{j}"], np.float32).reshape(NB * P, D)
        outp[m_off[e] + u0 * UNIT:m_off[e] + u0 * UNIT + nr] += seg[:nr]

    out = np.zeros((T + 1, D), np.float32)
    out[perm[:ltot]] = outp
    return out[:T]


# revision 6
# speedup vs baseline: 1.2313x; 1.1168x over previous
"""Grouped SwiGLU experts (MoE post-dispatch compute) on 8 Trainium2 cores.

Expert-parallel with optional hidden-dim tensor parallelism: host gathers
tokens per expert (the "all-to-all dispatch") and packs them into a uniform
per-core slot schedule (specialized to the actual counts at compile time).
A slot is a weight stream for one expert covering either the full hidden dim
(8 h-chunks) or half of it (4 h-chunks); half-width slots halve the weight
traffic per core and their partial outputs are summed on the host. Each slot
runs  hT = silu(w1.T x.T) * (w3.T x.T);  out = (hT.T) @ w2  in bf16 with
fp32 PSUM accumulation, and the host scatters rows back to token positions.
"""

import itertools
import numpy as np
import ml_dtypes

# ---- problem constants (from the reference module) ----
T, D, H, E, R, ALIGN = 8192, 4096, 1024, 8, 2, 16
P = 128          # partition width
DT = D // P      # 32 d-tiles
HT = H // P      # 8 h-chunks (full width)
NCORES = 8
UNIT = 16        # scheduling granularity in rows (reference ALIGN)
MAX_UNITS = 32   # max rows per slot = 512 (psum bank / f32 free-dim cap)

BF16 = ml_dtypes.bfloat16

# cost model for the planner
HU_NS = 2560          # compute ns per half-unit (16 rows x 384 cyc @2.4GHz)
PLAN_BW = 310.0       # planning DMA bandwidth, bytes/ns
W_FULL = 3 * D * H * 2            # full weight stream bytes (25.2MB)
W_HALF = W_FULL // 2
ROW_IO = 2 * D * 2                # x + out bytes per scheduled row (bf16)


def _permute_indices(counts):
    """numpy port of reference._permute_indices."""
    counts = counts.astype(np.int64)
    max_len = T + E * ALIGN
    start_index = np.cumsum(counts) - counts
    total = counts.reshape(R, E).sum(0)
    m_sizes = ((np.maximum(total, ALIGN) + ALIGN - 1) // ALIGN * ALIGN).astype(np.int64)
    m_offsets = np.cumsum(m_sizes)
    write_offsets = m_offsets - m_sizes
    c_er = counts.reshape(R, E).T
    seg_ws = (write_offsets[:, None] + np.cumsum(c_er, 1) - c_er).reshape(-1)
    seg_len = c_er.reshape(-1)
    seg_src = start_index.reshape(R, E).T.reshape(-1)
    pos = np.arange(max_len, dtype=np.int64)
    idx = np.clip(np.searchsorted(seg_ws, pos, side="right") - 1, 0, E * R - 1)
    within = pos - seg_ws[idx]
    valid = (within >= 0) & (within < seg_len[idx])
    perm = np.where(valid, seg_src[idx] + within, T)
    return perm.astype(np.int64), m_sizes, (m_offsets - m_sizes)


def _ffd(tasks, bins, strict):
    """Pack tasks (key, size) into bins [cap, core, j], splitting freely.
    Single task piece per bin. Returns (asg {(core,j): (key, u0, nu)},
    leftovers {key: units}); if strict, returns None on leftover."""
    bins = sorted(bins, key=lambda b: -b[0])
    used = [False] * len(bins)
    asg = {}
    left = {}
    for key, size in sorted(tasks, key=lambda t: -t[1]):
        rem = int(size)
        u0 = 0
        while rem > 0:
            best_le, best_gt = None, None
            for i, (cap, c, j) in enumerate(bins):
                if used[i]:
                    continue
                if cap <= rem:
                    best_le = i  # bins desc: first such is largest
                    break
                best_gt = i  # keeps updating: last seen = smallest so far
            i = best_le if best_le is not None else best_gt
            if i is None:
                if strict:
                    return None
                left[key] = rem
                break
            cap, c, j = bins[i]
            used[i] = True
            nb = min(cap, rem)
            asg[(c, j)] = (key, u0, nb)
            u0 += nb
            rem -= nb
    return asg, left


def _try_config(slots, nunits):
    """slots: [(units, nhc)]. Returns asg {(c,j): (e, half, u0, nu)} with
    half=None for full-width slots, or None if infeasible."""
    full_bins = [[u, c, j] for c in range(NCORES)
                 for j, (u, nhc) in enumerate(slots) if nhc == HT]
    half_bins = [[u, c, j] for c in range(NCORES)
                 for j, (u, nhc) in enumerate(slots) if nhc == HT // 2]

    def finish(asg_f, left):
        left = {e: r for e, r in left.items() if r > 0}
        if left and not half_bins:
            return None
        # leftover of expert e needs BOTH halves scheduled (split freely)
        half_tasks = [((e, h), r) for e, r in left.items() for h in (0, 1)]
        r = _ffd(half_tasks, half_bins, strict=True) if half_tasks else ({}, {})
        if r is None:
            return None
        asg_h, _ = r
        # full and half pieces of expert e cover disjoint unit ranges:
        # full pieces cover [0, nf_e), halves [nf_e, n_e)
        nf = {e: 0 for e in range(E)}
        for (c, j), (e, u0, nu) in asg_f.items():
            nf[e] = max(nf[e], u0 + nu)
        asg = {}
        for (c, j), (e, u0, nu) in asg_f.items():
            asg[(c, j)] = (e, None, u0, nu)
        for (c, j), ((e, h), u0, nu) in asg_h.items():
            asg[(c, j)] = (e, h, nf[e] + u0, nu)
        return asg

    if not full_bins:
        return finish({}, {e: int(nunits[e]) for e in range(E)})
    # which experts to route (primarily) through full-width bins is a small
    # subset-selection problem; greedy FFD alone misses exact packings
    fullcap = sum(b[0] for b in full_bins)
    subsets = []
    for mask in range(1 << E):
        S = [e for e in range(E) if mask >> e & 1]
        tot = sum(int(nunits[e]) for e in S)
        subsets.append((abs(tot - fullcap), mask, S))
    subsets.sort()
    for _, mask, S in subsets[:64]:
        tasks = [(e, int(nunits[e])) for e in S]
        asg_f, left = _ffd(tasks, full_bins, strict=False)
        for e in range(E):
            if not (mask >> e & 1):
                left[e] = int(nunits[e])
        asg = finish(asg_f, left)
        if asg is not None:
            return asg
    return None


def _cost(slots):
    chu = sum(u * (2 if nhc == HT else 1) for u, nhc in slots)
    rows = sum(u for u, _ in slots) * UNIT
    wbytes = sum(W_FULL if nhc == HT else W_HALF for _, nhc in slots)
    dma_ns = (wbytes + rows * ROW_IO) / PLAN_BW + 8000
    comp_ns = chu * HU_NS + 12000
    return max(comp_ns, dma_ns), dma_ns, len(slots)


def _plan(nunits):
    """Returns (slots [(units, nhc)], asg {(c,j): (e, half, u0, nu)})."""
    total_hu = 2 * int(sum(nunits))
    need = -(-total_hu // NCORES)
    best = None
    for extra_cap in (10, 40, 2 * need):
        for ns in (1, 2, 3):
            for widths in itertools.product((HT, HT // 2), repeat=ns):
                for us in itertools.product(range(1, MAX_UNITS + 1), repeat=ns):
                    cfg = tuple(sorted(zip(us, widths), key=lambda s: (-s[1], -s[0])))
                    if cfg != tuple(zip(us, widths)):
                        continue  # canonical order only
                    chu = sum(u * (2 if w == HT else 1) for u, w in cfg)
                    if not (need <= chu <= need + extra_cap):
                        continue
                    c = _cost(cfg)
                    if best is not None and c >= best[0]:
                        continue
                    asg = _try_config(cfg, nunits)
                    if asg is not None:
                        best = (c, list(cfg), asg)
        if best is not None:
            break
    if best is None:
        raise RuntimeError("packing failed")
    return best[1], best[2]


def _build_program(slots):
    import concourse.mybir as mybir
    import concourse.tile as tile
    from concourse import bacc

    bf = mybir.dt.bfloat16
    f32 = mybir.dt.float32
    SILU = mybir.ActivationFunctionType.Silu
    COPY = mybir.ActivationFunctionType.Copy

    nc = bacc.Bacc("TRN2", target_bir_lowering=False, debug=False,
                   num_devices=NCORES)

    XC = DT // 4  # d-tiles per xt chunk

    xt_d, w13_d, w2_d, out_d = [], [], [], []
    for j, (U, NHC) in enumerate(slots):
        M = U * UNIT
        xt_d.append(nc.dram_tensor(f"xt{j}", [4, P, XC * M], bf, kind="ExternalInput"))
        w13_d.append(nc.dram_tensor(f"w13p{j}", [2, NHC, P, D], bf, kind="ExternalInput"))
        w2_d.append(nc.dram_tensor(f"w2p{j}", [NHC, P, D], bf, kind="ExternalInput"))
        # output is stored transposed: out[k, p, t*M+m] = y[m, (k*XC+t)*P+p]
        out_d.append(nc.dram_tensor(f"out{j}", [4, P, XC * M], bf, kind="ExternalOutput"))
    wz_d = nc.dram_tensor("wz0", [P, P], bf, kind="ExternalInput")

    with tile.TileContext(nc) as tc:
        with (
            tc.tile_pool(name="xt", bufs=8) as xt_pool,
            tc.tile_pool(name="wp", bufs=6) as wp_pool,
            tc.tile_pool(name="w2", bufs=8) as w2_pool,
            tc.tile_pool(name="ht", bufs=12) as ht_pool,
            tc.tile_pool(name="stmp", bufs=2) as stmp_pool,
            tc.tile_pool(name="ost", bufs=2) as ost_pool,
            tc.tile_pool(name="ps1", bufs=2, space="PSUM") as ps1_pool,
            tc.tile_pool(name="ps3", bufs=2, space="PSUM") as ps3_pool,
            tc.tile_pool(name="pso", bufs=3, space="PSUM") as pso_pool,
            tc.tile_pool(name="warm", bufs=1) as warm_pool,
        ):
            # keep the PE busy (HAM at K=8/8) while the first real DMAs land
            wz = warm_pool.tile([P, P], bf, tag="warm", name="warmz")
            nc.sync.dma_start(out=wz[:], in_=wz_d[:, :])
            pw = pso_pool.tile([P, P], f32, tag="pso", name="warmp")
            for _ in range(140):
                nc.tensor.matmul(pw[:], wz[:], wz[:], start=True, stop=True)

            for j, (U, NHC) in enumerate(slots):
                M = U * UNIT

                def load_wpair(hc, j=j):
                    t1 = wp_pool.tile([P, D], bf, tag="wp", name=f"w1_{j}_{hc}")
                    nc.sync.dma_start(out=t1[:], in_=w13_d[j][0, hc])
                    t3 = wp_pool.tile([P, D], bf, tag="wp", name=f"w3_{j}_{hc}")
                    nc.sync.dma_start(out=t3[:], in_=w13_d[j][1, hc])
                    return (t1, t3)

                # critical path first: hc=0 weight panels, then token chunks,
                # then 2 more prefetched panel pairs (deep prefetch so the
                # in-order sync dispatcher never starves the PE);
                # w2 (phase 2 only) is deferred until after hc=1 emission
                wq = [load_wpair(0)]
                xts = []
                for k in range(4):
                    t = xt_pool.tile([P, XC * M], bf, tag="xt", name=f"xt{j}_{k}")
                    nc.sync.dma_start(out=t[:], in_=xt_d[j][k])
                    xts.append(t)
                for hc in (1, 2):
                    if hc < NHC:
                        wq.append(load_wpair(hc))
                hts = [ht_pool.tile([P, M], bf, tag="ht", name=f"ht{j}_{h}")
                       for h in range(NHC)]
                w2s = None
                for hc in range(NHC):
                    wcur = wq[hc]
                    if hc + 3 < NHC:
                        wq.append(load_wpair(hc + 3))
                    ps1 = ps1_pool.tile([P, M], f32, tag="ps1")
                    ps3 = ps3_pool.tile([P, M], f32, tag="ps3")
                    for d in range(DT):
                        xa = xts[d // XC][:, (d % XC) * M:(d % XC + 1) * M]
                        nc.tensor.matmul(ps1[:], wcur[0][:, d * P:(d + 1) * P],
                                         xa, start=(d == 0), stop=(d == DT - 1))
                        nc.tensor.matmul(ps3[:], wcur[1][:, d * P:(d + 1) * P],
                                         xa, start=(d == 0), stop=(d == DT - 1))
                    tmp = stmp_pool.tile([P, M], f32, tag="stmp")
                    nc.scalar.activation(tmp[:], ps1[:], SILU)
                    nc.vector.tensor_mul(hts[hc][:], tmp[:], ps3[:])
                    if hc == min(1, NHC - 1):
                        w2s = []
                        for h in range(NHC):
                            t = w2_pool.tile([P, D], bf, tag="w2",
                                             name=f"w2_{j}_{h}")
                            nc.sync.dma_start(out=t[:], in_=w2_d[j][h])
                            w2s.append(t)
                # phase 2, transposed: stationary w2 d-tile, stream tokens.
                # out psum is [d-cols, tokens]; no 128-row block padding.
                ob = None
                for d in range(DT):
                    pot = pso_pool.tile([P, M], f32, tag="pso")
                    for h in range(NHC):
                        nc.tensor.matmul(pot[:], w2s[h][:, d * P:(d + 1) * P],
                                         hts[h][:], start=(h == 0),
                                         stop=(h == NHC - 1))
                    if d % XC == 0:
                        ob = ost_pool.tile([P, XC * M], bf, tag="ost")
                    t = d % XC
                    # split psum->sbuf casts across scalar and vector
                    if d % 2 == 0:
                        nc.scalar.activation(ob[:, t * M:(t + 1) * M], pot[:], COPY)
                    else:
                        nc.vector.tensor_copy(ob[:, t * M:(t + 1) * M], pot[:])
                    if t == XC - 1:
                        nc.sync.dma_start(out=out_d[j][d // XC], in_=ob[:])

    nc.compile()
    return nc


_CACHE = {}


def _get_program(slots):
    key = tuple(slots)
    if key not in _CACHE:
        _CACHE[key] = _build_program(slots)
    return _CACHE[key]


_LAST_RESULT = None


def kernel(x, w1, w2, w3, num_tokens_per_expert):
    import os
    from concourse.bass_utils import run_bass_kernel_spmd

    x = np.asarray(x, dtype=np.float32)
    counts = np.asarray(num_tokens_per_expert).astype(np.int64)
    perm, m_sizes, m_off = _permute_indices(counts)
    nunits = m_sizes // UNIT  # m_sizes are UNIT-aligned

    slots, asg = _plan(nunits)
    nc = _get_program(slots)

    # expert-grouped token stream (the dispatch): rows of x per expert
    x_pad = np.concatenate([x, np.zeros((1, D), np.float32)], axis=0)
    ltot = int(m_sizes.sum())
    xp = x_pad[perm[:ltot]].astype(BF16)  # [ltot, D] expert-grouped
    xe = [xp[m_off[e]:m_off[e] + m_sizes[e]] for e in range(E)]

    w1b = [np.ascontiguousarray(
        np.asarray(w1[e], np.float32).reshape(DT, P, HT, P)
        .transpose(2, 1, 0, 3).reshape(HT, P, D)).astype(BF16) for e in range(E)]
    w3b = [np.ascontiguousarray(
        np.asarray(w3[e], np.float32).reshape(DT, P, HT, P)
        .transpose(2, 1, 0, 3).reshape(HT, P, D)).astype(BF16) for e in range(E)]
    w2b = [np.asarray(w2[e], np.float32).astype(BF16).reshape(HT, P, D)
           for e in range(E)]

    XC = DT // 4
    w13_cache = {}

    def w13_for(e, half, nhc):
        key = (e, half)
        if key not in w13_cache:
            off = 0 if half is None else half * nhc
            w13_cache[key] = np.stack([w1b[e][off:off + nhc],
                                       w3b[e][off:off + nhc]])
        return w13_cache[key]

    in_maps = []
    for c in range(NCORES):
        mm = {}
        for j, (U, NHC) in enumerate(slots):
            M = U * UNIT
            ent = asg.get((c, j))
            e, half = (ent[0], ent[1]) if ent is not None else (0, None if NHC == HT else 0)
            blk = np.zeros((M, D), BF16)
            if ent is not None:
                _, _, u0, nu = ent
                blk[:nu * UNIT] = xe[e][u0 * UNIT:(u0 + nu) * UNIT]
            # xt[k][p, t*M+m] = blk[m, (k*XC+t)*128+p]
            mm[f"xt{j}"] = np.ascontiguousarray(
                blk.reshape(M, 4, XC, P).transpose(1, 3, 2, 0).reshape(4, P, XC * M))
            mm[f"w13p{j}"] = w13_for(e, half, NHC)
            off = 0 if half is None else half * NHC
            mm[f"w2p{j}"] = w2b[e][off:off + NHC]
        mm["wz0"] = np.zeros((P, P), BF16)
        in_maps.append(mm)

    kw = {}
    if os.environ.get("KERNEL_TRACE"):
        kw = dict(trace=True, tmpdir=os.environ.get("KERNEL_TRACE_DIR") or None)
    res = run_bass_kernel_spmd(nc, in_maps, core_ids=list(range(NCORES)), **kw)
    global _LAST_RESULT
    _LAST_RESULT = res

    # reassemble expert-grouped output stream (summing half partials),
    # then scatter to token order
    outp = np.zeros((ltot, D), np.float32)
    for (c, j), (e, half, u0, nu) in asg.items():
        nr = nu * UNIT
        M = slots[j][0] * UNIT
        # out[k, p, t*M+m] = y[m, (k*XC+t)*P+p] -> [M, D]
        seg = np.asarray(res.results[c][f"out{j}"], np.float32) \
            .reshape(4, P, XC, M).transpose(3, 0, 2, 1).reshape(M, D)
        outp[m_off[e] + u0 * UNIT:m_off[e] + u0 * UNIT + nr] += seg[:nr]

    out = np.zeros((T + 1, D), np.float32)
    out[perm[:ltot]] = outp
    return out[:T]
